# revision 10
# baseline (speedup 1.0000x reference)
"""CIN (Compressed Interaction Network) forward pass on 8 Trainium2 cores.

Math (per layer k, per batch b):
    x_{k+1}[b, l, d] = sum_{m, h} x[b, m, d] * x_k[b, h, d] * W_k[m, h, l]
    p_k[b, l]        = sum_d x_{k+1}[b, l, d]
Output: concat(p_0, p_1, p_2) -> [B, 384].

Sharding: data-parallel over batch (2048 -> 8 x 256), weights replicated.

Per-core kernel layout: batches processed in groups of G=8 -> free-dim
columns c = (b, d) with N = G*64 = 512.  For each group:
  BX[m][p, c] = x[b, m, d]  (x rows broadcast across partitions via DMA)
  layer k:  z_m = XK (.) BX[m]  (DVE), acc[l, c] += W_k[m]^T @ z_m  (PE,
  float32r full-rate matmuls accumulating in one PSUM bank)
  p_k = reduce_d(acc)  -> [128 l, 8 b]
Final: PE-transpose p tiles -> [b, l] and contiguous DMA out.
"""

import os
import sys

import numpy as np

sys.path.insert(0, "/opt/trn_rl_repo")

B, M, D = 2048, 40, 64
L = 128  # all three CIN layer widths
N_CORES = 8
B_LOCAL = B // N_CORES  # 256

_BUILT = None


def _build(b_local: int = B_LOCAL, mm_dt_name: str = "float32r", groups_per_iter: int | None = None):
    """Build the Bass module for one core processing b_local batches."""
    from contextlib import ExitStack

    import concourse.bass as bass
    import concourse.mybir as mybir
    from concourse.alu_op_type import AluOpType
    from bass_rust import AxisListType
    from concourse.masks import make_identity
    from concourse.tile import TileContext

    f32 = mybir.dt.float32
    mm_dt = getattr(mybir.dt, mm_dt_name)
    G = 8  # batches per group
    N = G * D  # 512 free columns per group
    n_groups = b_local // G
    MQ = 4  # m values per z-quad
    NQ = M // MQ  # 10 quads

    from concourse import bacc

    nc = bacc.Bacc(None, target_bir_lowering=False)
    x = nc.dram_tensor("x", [b_local, M, D], f32, kind="ExternalInput")
    w0 = nc.dram_tensor("W0", [M, M, L], f32, kind="ExternalInput")
    w1 = nc.dram_tensor("W1", [M, L, L], f32, kind="ExternalInput")
    w2 = nc.dram_tensor("W2", [M, L, L], f32, kind="ExternalInput")
    out = nc.dram_tensor("out", [b_local, 3 * L], f32, kind="ExternalOutput")

    with TileContext(nc) as tc, ExitStack() as ctx:
        singles = ctx.enter_context(tc.tile_pool(name="singles", bufs=1))
        xh_pool = ctx.enter_context(tc.tile_pool(name="xh", bufs=2))
        bx_pool = ctx.enter_context(tc.tile_pool(name="bx", bufs=1))
        z_pool = ctx.enter_context(tc.tile_pool(name="z", bufs=3))
        xk_pool = ctx.enter_context(tc.tile_pool(name="xk", bufs=4))
        psum_pool = ctx.enter_context(tc.tile_pool(name="psum", bufs=3, space="PSUM"))
        tp_pool = ctx.enter_context(tc.tile_pool(name="tpsum", bufs=2, space="PSUM"))

        # --- persistent weights, [h, (m l)] so lhsT slice for m is [h, 128]
        # (transposed load: partition = h, free = (m, l))
        wp0 = singles.tile([M, M * L], mm_dt, tag="wp0")
        nc.gpsimd.dma_start(
            out=wp0[:],
            in_=bass.AP(tensor=w0, offset=0, ap=[[L, M], [M * L, M], [1, L]]),
        )
        wp1 = singles.tile([L, M * L], mm_dt, tag="wp1")
        nc.gpsimd.dma_start(
            out=wp1[:],
            in_=bass.AP(tensor=w1, offset=0, ap=[[L, L], [L * L, M], [1, L]]),
        )
        wp2 = singles.tile([L, M * L], mm_dt, tag="wp2")
        nc.gpsimd.dma_start(
            out=wp2[:],
            in_=bass.AP(tensor=w2, offset=0, ap=[[L, L], [L * L, M], [1, L]]),
        )

        ident = singles.tile([128, 128], f32, tag="ident")
        make_identity(nc, ident[:])

        # p accumulators: [128 l, (layer, b_local)]
        pl = singles.tile([L, 3 * b_local], f32, tag="pl")

        for g in range(n_groups):
            b0 = g * G
            # x rows in h-layout: XH[m, (b, d)]
            xh = xh_pool.tile([M, N], f32, tag="xh")
            nc.sync.dma_start(
                out=xh[:],
                in_=bass.AP(
                    tensor=x,
                    offset=b0 * M * D,
                    ap=[[D, M], [M * D, G], [1, D]],
                ),
            )
            # broadcast tiles: BX[:, m*N + c] = x[b, m, d] for all partitions.
            # DMA direct from DRAM with a step-0 partition dim.
            bx = bx_pool.tile([128, M * N], f32, tag="bx")
            for m in range(M):
                src = bass.AP(
                    tensor=x,
                    offset=(b0 * M + m) * D,
                    ap=[[0, 128], [M * D, G], [1, D]],
                )
                nc.sync.dma_start(out=bx[:, m * N : (m + 1) * N], in_=src)

            xk = xh  # layer 0 contracts against x itself (h = m, 40 rows)
            for layer, (wp, kdim) in enumerate([(wp0, M), (wp1, L), (wp2, L)]):
                acc = psum_pool.tile([128, N], f32, tag="acc")
                for q in range(NQ):
                    z = z_pool.tile([128, MQ * N], mm_dt, tag="z")
                    zv = z[:kdim].rearrange("p (m n) -> p m n", n=N)
                    nc.vector.tensor_tensor(
                        out=zv,
                        in0=xk[:kdim].unsqueeze(1).broadcast_to([kdim, MQ, N]),
                        in1=bx[:kdim, q * MQ * N : (q + 1) * MQ * N].rearrange(
                            "p (m n) -> p m n", n=N
                        ),
                        op=AluOpType.mult,
                    )
                    for j in range(MQ):
                        m = q * MQ + j
                        nc.tensor.matmul(
                            acc[:],
                            lhsT=wp[:, m * L : (m + 1) * L],
                            rhs=z[:kdim, j * N : (j + 1) * N],
                            start=(m == 0),
                            stop=(m == M - 1),
                        )
                if layer < 2:
                    xk_new = xk_pool.tile([L, N], f32, tag="xk")
                    nc.scalar.copy(out=xk_new[:], in_=acc[:])
                    nc.vector.reduce_sum(
                        out=pl[:, layer * b_local + b0 : layer * b_local + b0 + G],
                        in_=xk_new[:].rearrange("p (b d) -> p b d", d=D),
                        axis=AxisListType.X,
                    )
                    xk = xk_new
                else:
                    nc.vector.reduce_sum(
                        out=pl[:, layer * b_local + b0 : layer * b_local + b0 + G],
                        in_=acc[:].rearrange("p (b d) -> p b d", d=D),
                        axis=AxisListType.X,
                    )

        # --- transpose p: [128 l, b] -> [b, l] tiles, then contiguous DMA out
        n_btiles = (b_local + 127) // 128
        for bt in range(n_btiles):
            bw = min(128, b_local - bt * 128)
            pt = singles.tile([128, 3 * L], f32, tag=f"pt{bt}")
            for layer in range(3):
                tp = tp_pool.tile([128, 128], f32, tag="tp")
                nc.tensor.transpose(
                    tp[:bw],
                    pl[:, layer * b_local + bt * 128 : layer * b_local + bt * 128 + bw],
                    ident[:],
                )
                nc.scalar.copy(out=pt[:bw, layer * L : (layer + 1) * L], in_=tp[:bw])
            nc.sync.dma_start(
                out=out[bt * 128 : bt * 128 + bw, :], in_=pt[:bw]
            )

    nc.finalize()
    return nc


def _get_built():
    global _BUILT
    if _BUILT is None:
        _BUILT = _build()
    return _BUILT


def kernel(**inputs: np.ndarray) -> np.ndarray:
    from concourse import bass_utils

    x = np.ascontiguousarray(inputs["x"], dtype=np.float32)
    w0 = np.ascontiguousarray(inputs["W0"], dtype=np.float32)
    w1 = np.ascontiguousarray(inputs["W1"], dtype=np.float32)
    w2 = np.ascontiguousarray(inputs["W2"], dtype=np.float32)

    nc = _get_built()
    in_maps = []
    for i in range(N_CORES):
        shard = np.ascontiguousarray(x[i * B_LOCAL : (i + 1) * B_LOCAL])
        in_maps.append({"x": shard, "W0": w0, "W1": w1, "W2": w2})

    trace = bool(int(os.environ.get("CIN_TRACE", "0")))
    res = bass_utils.run_bass_kernel_spmd(
        nc, in_maps, core_ids=list(range(N_CORES)), trace=trace
    )
    if trace:
        kernel.last_results = res
    return np.concatenate([r["out"] for r in res.results], axis=0)
